# revision 22
# baseline (speedup 1.0000x reference)
"""MoE top-2 routing kernel for 8 Trainium2 NeuronCores.

Strategy (expert-parallel with host-side dispatch):
  - Router (x @ w_router, softmax, top-2, combine weights) computed on host:
    it is 0.1% of the FLOPs and produces the dispatch indices needed to
    shard the tokens anyway.
  - Each of the 6 experts' FFN (D=1024 -> H=4096 -> D=1024) is split 4-ways
    along the hidden dim H into 24 shards of (1024 -> 1024 -> 1024).
    24 shards / 8 cores = 3 shards (slots) per core, perfectly
    weight-balanced. Experts are sorted by routed-token count and packed two
    per slot so each slot's capacity is the max of just those two experts
    (not the global max), minimizing zero-padding.
  - Tokens routed to expert e (gathered, transposed to [D, W] feature-major,
    zero-padded to the slot capacity W) are processed by all 4 of e's
    shards; each shard produces a partial y^T[D, W] (sum over its H
    quarter). Host sums the 4 partials per expert, scales by the top-2
    combine weight and scatter-adds into the output.
  - Device kernel per core: 3x dense fused MLP: h^T = gelu(w1s^T x^T) tile
    by tile, y^T = w2s^T h^T. Data is float32 in HBM but fed to the PE as
    float32r (single-pass ~fp22 matmul, fp32 PSUM accumulate) — the default
    float32 matmul mode (LOW_HIGH) is 2 passes and ~3x slower.

Measured on trn2 (8 cores): ~500 us HW exec, out rel-err ~2.4e-4 vs the
fp32 jax reference. PE streams at ~213-227 ns per 128x128x512 matmul in
steady state, ~85% of the dense-PE roofline for the routed FLOPs.
"""

import functools
import sys
import time
import types

import numpy as np

# bass_utils' axon trace path does `from antenv.axon_hooks import ...`; some
# images ship an antenv without that module, which would turn a BASS_TRACE=1
# run into an ImportError. Register a no-op stand-in so tracing degrades
# gracefully instead (run_bass_kernel_spmd skips tracing on a None hook).
try:
    import antenv.axon_hooks  # noqa: F401
except Exception:
    try:
        import antenv  # noqa: F401
        _hooks = types.ModuleType("antenv.axon_hooks")
        _hooks._HOOK = None
        _hooks.set_axon_ntff_profile_hook = (
            lambda hook: setattr(_hooks, "_HOOK", hook)
        )
        _hooks.get_axon_ntff_profile_hook = lambda: _hooks._HOOK
        sys.modules["antenv.axon_hooks"] = _hooks
        antenv.axon_hooks = _hooks
    except Exception:
        pass

import concourse.bacc as bacc
import concourse.bass as bass
import concourse.mybir as mybir
import concourse.tile as tile
from concourse.bass_utils import run_bass_kernel_spmd

N_EXPERTS = 6
TOP_K = 2
AUX_COEFF = 0.01
B, T, D, H = 4, 2048, 1024, 4096
N_TOKENS = B * T
N_CORES = 8
N_SPLIT = 4                       # H split per expert
H_SH = H // N_SPLIT               # 1024
S_PER_CORE = 3                    # expert-shard slots per core
P = 128
FREE = 512                        # matmul moving free dim / PSUM bank width

# Populated by kernel() with the BassKernelResults of the last device run so
# a test harness can read exec_time_ns when BASS_TRACE=1 is set.
LAST_RESULTS = None


def _tile_widths(W: int) -> list:
    widths = [FREE] * (W // FREE)
    if W % FREE:
        widths.append(W % FREE)
    return widths


@functools.cache
def _build(slot_widths: tuple) -> bass.Bass:
    """Bass program for one core: 3 independent (1024 -> 1024 -> 1024) dense
    MLP shards, slot j over slot_widths[j] tokens, float32r feature-major."""
    f32 = mybir.dt.float32
    f32r = mybir.dt.float32r
    KD = D // P      # 8 contraction chunks for layer 1
    KH = H_SH // P   # 8 contraction chunks for layer 2

    nc = bacc.Bacc()
    xTs = [nc.dram_tensor(f"xT{j}", [D, W], f32r, kind="ExternalInput")
           for j, W in enumerate(slot_widths)]
    w1s = nc.dram_tensor("w1s", [S_PER_CORE, D, H_SH], f32r, kind="ExternalInput")
    w2s = nc.dram_tensor("w2s", [S_PER_CORE, H_SH, D], f32r, kind="ExternalInput")
    yTs = [nc.dram_tensor(f"yT{j}", [D, W], f32r, kind="ExternalOutput")
           for j, W in enumerate(slot_widths)]

    with tile.TileContext(nc) as tc:
        with (
            tc.tile_pool(name="w1p", bufs=3 * KD) as w1p,
            tc.tile_pool(name="w2p", bufs=KH + KH // 2) as w2p,
            tc.tile_pool(name="xp", bufs=2 * KD + 2) as xp,
            tc.tile_pool(name="hp", bufs=2 * KH + 2) as hp,
            tc.tile_pool(name="yp", bufs=4) as yp,
            tc.tile_pool(name="psh", bufs=4, space="PSUM") as psh,
            tc.tile_pool(name="psy", bufs=4, space="PSUM") as psy,
        ):
            def load_x(s, n, nsl, nw):
                xt = []
                for k in range(KD):
                    xk = xp.tile([P, FREE], f32r, tag="x", name=f"x_{s}_{n}_{k}")
                    nc.sync.dma_start(xk[:, :nw], xTs[s][k * P:(k + 1) * P, nsl])
                    xt.append(xk)
                return xt

            for s in range(S_PER_CORE):
                widths = _tile_widths(slot_widths[s])
                if s == 0 and slot_widths[0] >= 2 * FREE:
                    # Narrow first pair so the very first matmuls wait on
                    # ~3 MB of DMA (w1 low halves + 1 MB of x) instead of 4.
                    widths = [FREE // 2, FREE // 2] + _tile_widths(slot_widths[0] - FREE)
                # Pair the n-tiles: two subtiles share one weight slice per
                # (m, k), halving LDWEIGHTS pressure on the PE. Each pair is
                # (col offset a, width a, col offset b | None, width b).
                offs = [0]
                for w in widths:
                    offs.append(offs[-1] + w)
                pairs = []
                i = 0
                while i < len(widths):
                    if i + 1 < len(widths):
                        pairs.append((offs[i], widths[i], offs[i + 1], widths[i + 1]))
                        i += 2
                    else:
                        pairs.append((offs[i], widths[i], None, 0))
                        i += 1

                # w1 loaded in half-width tiles, low halves first: the first
                # matmuls (m=0..3) need only cols 0:512 of each k chunk, so
                # compute starts after 2 MB instead of 4 MB of weight DMA.
                w1t = [[None, None] for _ in range(KD)]

                def load_w1_half(s, half):
                    csl = slice(half * (H_SH // 2), (half + 1) * (H_SH // 2))
                    for k in range(KD):
                        w1k = w1p.tile([P, H_SH // 2], f32r, tag="w1",
                                       name=f"w1_{s}_{k}_{half}")
                        nc.sync.dma_start(w1k[:], w1s[s, k * P:(k + 1) * P, csl])
                        w1t[k][half] = w1k

                # DMA issue order tracks first-use order: w1 low halves
                # (m=0..3), first n-pair's x, w1 high halves, then w2
                # (layer 2 starts ~15 us after layer 1).
                load_w1_half(s, 0)
                ca0, nwa0, cb0, nwb0 = pairs[0]
                xta0 = load_x(s, ca0, slice(ca0, ca0 + nwa0), nwa0)
                xtb0 = (load_x(s, cb0, slice(cb0, cb0 + nwb0), nwb0)
                        if cb0 is not None else None)
                load_w1_half(s, 1)
                w2t = []
                for k in range(KH):
                    w2k = w2p.tile([P, D], f32r, tag="w2", name=f"w2_{s}_{k}")
                    nc.sync.dma_start(w2k[:], w2s[s, k * P:(k + 1) * P, :])
                    w2t.append(w2k)

                for pi, (ca, nwa, cb, nwb) in enumerate(pairs):
                    na = ca
                    sla = slice(ca, ca + nwa)
                    slb = slice(cb, cb + nwb) if cb is not None else None
                    if pi == 0:
                        xta, xtb = xta0, xtb0
                    else:
                        xta = load_x(s, ca, sla, nwa)
                        xtb = load_x(s, cb, slb, nwb) if cb is not None else None
                    hta, htb = [], []
                    for m in range(KH):
                        pha = psh.tile([P, FREE], f32, tag="ps_h", name=f"pha_{s}_{na}_{m}")
                        phb = (psh.tile([P, FREE], f32, tag="ps_h", name=f"phb_{s}_{na}_{m}")
                               if xtb is not None else None)
                        for k in range(KD):
                            w1sl = w1t[k][m // 4][:, (m % 4) * P:(m % 4 + 1) * P]
                            nc.tensor.matmul(pha[:, :nwa], w1sl, xta[k][:, :nwa],
                                             start=(k == 0), stop=(k == KD - 1))
                            if phb is not None:
                                nc.tensor.matmul(phb[:, :nwb], w1sl, xtb[k][:, :nwb],
                                                 start=(k == 0), stop=(k == KD - 1))
                        hma = hp.tile([P, FREE], f32r, tag="h", name=f"hma_{s}_{na}_{m}")
                        nc.scalar.activation(hma[:, :nwa], pha[:, :nwa],
                                             mybir.ActivationFunctionType.Gelu)
                        hta.append(hma)
                        if phb is not None:
                            hmb = hp.tile([P, FREE], f32r, tag="h", name=f"hmb_{s}_{na}_{m}")
                            nc.scalar.activation(hmb[:, :nwb], phb[:, :nwb],
                                                 mybir.ActivationFunctionType.Gelu)
                            htb.append(hmb)
                    for d in range(KD):
                        pya = psy.tile([P, FREE], f32, tag="ps_y", name=f"pya_{s}_{na}_{d}")
                        pyb = (psy.tile([P, FREE], f32, tag="ps_y", name=f"pyb_{s}_{na}_{d}")
                               if xtb is not None else None)
                        for k in range(KH):
                            w2sl = w2t[k][:, d * P:(d + 1) * P]
                            nc.tensor.matmul(pya[:, :nwa], w2sl, hta[k][:, :nwa],
                                             start=(k == 0), stop=(k == KH - 1))
                            if pyb is not None:
                                nc.tensor.matmul(pyb[:, :nwb], w2sl, htb[k][:, :nwb],
                                                 start=(k == 0), stop=(k == KH - 1))
                        yda = yp.tile([P, FREE], f32r, tag="y", name=f"yda_{s}_{na}_{d}")
                        nc.vector.tensor_copy(yda[:, :nwa], pya[:, :nwa])
                        nc.sync.dma_start(yTs[s][d * P:(d + 1) * P, sla], yda[:, :nwa])
                        if pyb is not None:
                            ydb = yp.tile([P, FREE], f32r, tag="y", name=f"ydb_{s}_{na}_{d}")
                            nc.vector.tensor_copy(ydb[:, :nwb], pyb[:, :nwb])
                            nc.sync.dma_start(yTs[s][d * P:(d + 1) * P, slb], ydb[:, :nwb])
    nc.finalize()
    return nc


def _route(xf: np.ndarray, w_router: np.ndarray):
    """Host router: softmax probs (float64 for stable ordering), top-2
    indices and renormalized combine weights, aux loss."""
    logits = xf.astype(np.float64) @ w_router.astype(np.float64)
    z = logits - logits.max(axis=-1, keepdims=True)
    p = np.exp(z)
    p /= p.sum(axis=-1, keepdims=True)

    ar = np.arange(xf.shape[0])
    top1 = p.argmax(axis=-1)
    pm = p.copy()
    pm[ar, top1] = -np.inf
    top2 = pm.argmax(axis=-1)
    p1 = p[ar, top1]
    p2 = p[ar, top2]
    c1 = p1 / (p1 + p2)
    c2 = p2 / (p1 + p2)

    tokens_per_expert = p.mean(axis=0)
    aux = AUX_COEFF * np.mean((tokens_per_expert - 1.0 / N_EXPERTS) ** 2)
    return top1, top2, c1, c2, np.float32(aux)


def kernel(x, w_router, w1, w2):
    global LAST_RESULTS
    x = np.asarray(x, dtype=np.float32)
    w_router = np.asarray(w_router, dtype=np.float32)
    w1 = np.asarray(w1, dtype=np.float32)
    w2 = np.asarray(w2, dtype=np.float32)

    xf = x.reshape(N_TOKENS, D)
    top1, top2, c1, c2, aux = _route(xf, w_router)

    # Gather tokens per expert.
    idx = [np.where((top1 == e) | (top2 == e))[0] for e in range(N_EXPERTS)]
    comb = [
        np.where(top1[idx[e]] == e, c1[idx[e]], c2[idx[e]]).astype(np.float32)
        for e in range(N_EXPERTS)
    ]
    counts = [len(i) for i in idx]

    # Sort experts by load; slot j serves expert rank 2j + (core // 4) with
    # hidden quarter (core % 4). Slot capacity = max count of its 2 experts,
    # rounded to 16 elements (64 B DMA row alignment).
    eorder = sorted(range(N_EXPERTS), key=lambda e: -counts[e])
    slot_widths = []
    for j in range(S_PER_CORE):
        w = max(counts[eorder[2 * j]], counts[eorder[2 * j + 1]], 1)
        slot_widths.append(max(128, -(-w // 16) * 16))
    slot_widths = tuple(slot_widths)

    xT_e = {}
    for j in range(S_PER_CORE):
        for g in range(2):
            e = eorder[2 * j + g]
            gbuf = np.zeros((D, slot_widths[j]), dtype=np.float32)
            gbuf[:, :counts[e]] = xf[idx[e]].T
            xT_e[e] = gbuf

    in_maps = []
    for core in range(N_CORES):
        g, q = divmod(core, N_SPLIT)
        w1c = np.empty((S_PER_CORE, D, H_SH), dtype=np.float32)
        w2c = np.empty((S_PER_CORE, H_SH, D), dtype=np.float32)
        im = {}
        for j in range(S_PER_CORE):
            e = eorder[2 * j + g]
            im[f"xT{j}"] = xT_e[e]
            w1c[j] = w1[e][:, q * H_SH:(q + 1) * H_SH]
            w2c[j] = w2[e][q * H_SH:(q + 1) * H_SH, :]
        im["w1s"] = w1c
        im["w2s"] = w2c
        in_maps.append(im)

    nc = _build(slot_widths)
    res = None
    for attempt in range(3):
        try:
            res = run_bass_kernel_spmd(nc, in_maps, core_ids=list(range(N_CORES)))
            break
        except Exception:
            if attempt == 2:
                raise
            time.sleep(5.0)
    LAST_RESULTS = res

    out = np.zeros((N_TOKENS, D), dtype=np.float32)
    for j in range(S_PER_CORE):
        for g in range(2):
            e = eorder[2 * j + g]
            acc = np.zeros((D, counts[e]), dtype=np.float32)
            for q in range(N_SPLIT):
                core = g * N_SPLIT + q
                acc += res.results[core][f"yT{j}"][:, :counts[e]]
            out[idx[e]] += comb[e][:, None] * acc.T

    return out.reshape(B, T, D), aux


# revision 23
# speedup vs baseline: 1.0372x; 1.0372x over previous
"""MoE top-2 routing kernel for 8 Trainium2 NeuronCores.

Strategy (expert-parallel with host-side dispatch):
  - Router (x @ w_router, softmax, top-2, combine weights) computed on host:
    it is 0.1% of the FLOPs and produces the dispatch indices needed to
    shard the tokens anyway.
  - Each of the 6 experts' FFN (D=1024 -> H=4096 -> D=1024) is split 4-ways
    along the hidden dim H into 24 shards of (1024 -> 1024 -> 1024).
    24 shards / 8 cores = 3 shards (slots) per core, perfectly
    weight-balanced. Experts are sorted by routed-token count and packed two
    per slot so each slot's capacity is the max of just those two experts
    (not the global max), minimizing zero-padding.
  - Tokens routed to expert e (gathered, transposed to [D, W] feature-major,
    zero-padded to the slot capacity W) are processed by all 4 of e's
    shards; each shard produces a partial y^T[D, W] (sum over its H
    quarter). Host sums the 4 partials per expert, scales by the top-2
    combine weight and scatter-adds into the output.
  - Device kernel per core: 3x dense fused MLP: h^T = gelu(w1s^T x^T) tile
    by tile, y^T = w2s^T h^T. Data is float32 in HBM but fed to the PE as
    float32r (single-pass ~fp22 matmul, fp32 PSUM accumulate) — the default
    float32 matmul mode (LOW_HIGH) is 2 passes and ~3x slower.

Measured on trn2 (8 cores): ~500 us HW exec, out rel-err ~2.4e-4 vs the
fp32 jax reference. PE streams at ~213-227 ns per 128x128x512 matmul in
steady state, ~85% of the dense-PE roofline for the routed FLOPs.
"""

import functools
import sys
import time
import types

import numpy as np

# bass_utils' axon trace path does `from antenv.axon_hooks import ...`; some
# images ship an antenv without that module, which would turn a BASS_TRACE=1
# run into an ImportError. Register a no-op stand-in so tracing degrades
# gracefully instead (run_bass_kernel_spmd skips tracing on a None hook).
try:
    import antenv.axon_hooks  # noqa: F401
except Exception:
    try:
        import antenv  # noqa: F401
        _hooks = types.ModuleType("antenv.axon_hooks")
        _hooks._HOOK = None
        _hooks.set_axon_ntff_profile_hook = (
            lambda hook: setattr(_hooks, "_HOOK", hook)
        )
        _hooks.get_axon_ntff_profile_hook = lambda: _hooks._HOOK
        sys.modules["antenv.axon_hooks"] = _hooks
        antenv.axon_hooks = _hooks
    except Exception:
        pass

import concourse.bacc as bacc
import concourse.bass as bass
import concourse.mybir as mybir
import concourse.tile as tile
from concourse.bass_utils import run_bass_kernel_spmd

N_EXPERTS = 6
TOP_K = 2
AUX_COEFF = 0.01
B, T, D, H = 4, 2048, 1024, 4096
N_TOKENS = B * T
N_CORES = 8
N_SPLIT = 4                       # H split per expert
H_SH = H // N_SPLIT               # 1024
S_PER_CORE = 3                    # expert-shard slots per core
P = 128
FREE = 512                        # matmul moving free dim / PSUM bank width

# Populated by kernel() with the BassKernelResults of the last device run so
# a test harness can read exec_time_ns when BASS_TRACE=1 is set.
LAST_RESULTS = None


def _tile_widths(W: int) -> list:
    widths = [FREE] * (W // FREE)
    if W % FREE:
        widths.append(W % FREE)
    return widths


@functools.cache
def _build(slot_widths: tuple) -> bass.Bass:
    """Bass program for one core: 3 independent (1024 -> 1024 -> 1024) dense
    MLP shards, slot j over slot_widths[j] tokens, float32r feature-major."""
    f32 = mybir.dt.float32
    f32r = mybir.dt.float32r
    KD = D // P      # 8 contraction chunks for layer 1
    KH = H_SH // P   # 8 contraction chunks for layer 2

    nc = bacc.Bacc()
    xTs = [nc.dram_tensor(f"xT{j}", [D, W], f32r, kind="ExternalInput")
           for j, W in enumerate(slot_widths)]
    w1s = nc.dram_tensor("w1s", [S_PER_CORE, D, H_SH], f32r, kind="ExternalInput")
    w2s = nc.dram_tensor("w2s", [S_PER_CORE, H_SH, D], f32r, kind="ExternalInput")
    yTs = [nc.dram_tensor(f"yT{j}", [D, W], f32r, kind="ExternalOutput")
           for j, W in enumerate(slot_widths)]

    with tile.TileContext(nc) as tc:
        with (
            tc.tile_pool(name="w1p", bufs=3 * KD) as w1p,
            tc.tile_pool(name="w2p", bufs=KH + KH // 2) as w2p,
            tc.tile_pool(name="xp", bufs=2 * KD + 2) as xp,
            tc.tile_pool(name="hp", bufs=2 * KH + 2) as hp,
            tc.tile_pool(name="yp", bufs=4) as yp,
            tc.tile_pool(name="psh", bufs=4, space="PSUM") as psh,
            tc.tile_pool(name="psy", bufs=4, space="PSUM") as psy,
        ):
            def load_x(s, n, nsl, nw):
                xt = []
                for k in range(KD):
                    xk = xp.tile([P, FREE], f32r, tag="x", name=f"x_{s}_{n}_{k}")
                    nc.sync.dma_start(xk[:, :nw], xTs[s][k * P:(k + 1) * P, nsl])
                    xt.append(xk)
                return xt

            for s in range(S_PER_CORE):
                widths = _tile_widths(slot_widths[s])
                # Pair the n-tiles: two subtiles share one weight slice per
                # (m, k), halving LDWEIGHTS pressure on the PE. Each pair is
                # (col offset a, width a, col offset b | None, width b).
                offs = [0]
                for w in widths:
                    offs.append(offs[-1] + w)
                pairs = []
                i = 0
                while i < len(widths):
                    if i + 1 < len(widths):
                        pairs.append((offs[i], widths[i], offs[i + 1], widths[i + 1]))
                        i += 2
                    else:
                        pairs.append((offs[i], widths[i], None, 0))
                        i += 1

                # w1 loaded in half-width tiles, low halves first: the first
                # matmuls (m=0..3) need only cols 0:512 of each k chunk, so
                # compute starts after 2 MB instead of 4 MB of weight DMA.
                w1t = [[None, None] for _ in range(KD)]

                def load_w1_half(s, half):
                    csl = slice(half * (H_SH // 2), (half + 1) * (H_SH // 2))
                    for k in range(KD):
                        w1k = w1p.tile([P, H_SH // 2], f32r, tag="w1",
                                       name=f"w1_{s}_{k}_{half}")
                        nc.sync.dma_start(w1k[:], w1s[s, k * P:(k + 1) * P, csl])
                        w1t[k][half] = w1k

                # DMA issue order tracks first-use order: w1 low halves
                # (m=0..3), first n-pair's x, w1 high halves, then w2
                # (layer 2 starts ~15 us after layer 1).
                load_w1_half(s, 0)
                ca0, nwa0, cb0, nwb0 = pairs[0]
                xta0 = load_x(s, ca0, slice(ca0, ca0 + nwa0), nwa0)
                xtb0 = (load_x(s, cb0, slice(cb0, cb0 + nwb0), nwb0)
                        if cb0 is not None else None)
                load_w1_half(s, 1)
                w2t = []
                for k in range(KH):
                    w2k = w2p.tile([P, D], f32r, tag="w2", name=f"w2_{s}_{k}")
                    nc.sync.dma_start(w2k[:], w2s[s, k * P:(k + 1) * P, :])
                    w2t.append(w2k)

                for pi, (ca, nwa, cb, nwb) in enumerate(pairs):
                    na = ca
                    sla = slice(ca, ca + nwa)
                    slb = slice(cb, cb + nwb) if cb is not None else None
                    if pi == 0:
                        xta, xtb = xta0, xtb0
                    else:
                        xta = load_x(s, ca, sla, nwa)
                        xtb = load_x(s, cb, slb, nwb) if cb is not None else None
                    hta, htb = [], []
                    for m in range(KH):
                        pha = psh.tile([P, FREE], f32, tag="ps_h", name=f"pha_{s}_{na}_{m}")
                        phb = (psh.tile([P, FREE], f32, tag="ps_h", name=f"phb_{s}_{na}_{m}")
                               if xtb is not None else None)
                        for k in range(KD):
                            w1sl = w1t[k][m // 4][:, (m % 4) * P:(m % 4 + 1) * P]
                            nc.tensor.matmul(pha[:, :nwa], w1sl, xta[k][:, :nwa],
                                             start=(k == 0), stop=(k == KD - 1))
                            if phb is not None:
                                nc.tensor.matmul(phb[:, :nwb], w1sl, xtb[k][:, :nwb],
                                                 start=(k == 0), stop=(k == KD - 1))
                        hma = hp.tile([P, FREE], f32r, tag="h", name=f"hma_{s}_{na}_{m}")
                        nc.scalar.activation(hma[:, :nwa], pha[:, :nwa],
                                             mybir.ActivationFunctionType.Gelu)
                        hta.append(hma)
                        if phb is not None:
                            hmb = hp.tile([P, FREE], f32r, tag="h", name=f"hmb_{s}_{na}_{m}")
                            nc.scalar.activation(hmb[:, :nwb], phb[:, :nwb],
                                                 mybir.ActivationFunctionType.Gelu)
                            htb.append(hmb)
                    for d in range(KD):
                        pya = psy.tile([P, FREE], f32, tag="ps_y", name=f"pya_{s}_{na}_{d}")
                        pyb = (psy.tile([P, FREE], f32, tag="ps_y", name=f"pyb_{s}_{na}_{d}")
                               if xtb is not None else None)
                        for k in range(KH):
                            w2sl = w2t[k][:, d * P:(d + 1) * P]
                            nc.tensor.matmul(pya[:, :nwa], w2sl, hta[k][:, :nwa],
                                             start=(k == 0), stop=(k == KH - 1))
                            if pyb is not None:
                                nc.tensor.matmul(pyb[:, :nwb], w2sl, htb[k][:, :nwb],
                                                 start=(k == 0), stop=(k == KH - 1))
                        yda = yp.tile([P, FREE], f32r, tag="y", name=f"yda_{s}_{na}_{d}")
                        nc.vector.tensor_copy(yda[:, :nwa], pya[:, :nwa])
                        nc.sync.dma_start(yTs[s][d * P:(d + 1) * P, sla], yda[:, :nwa])
                        if pyb is not None:
                            ydb = yp.tile([P, FREE], f32r, tag="y", name=f"ydb_{s}_{na}_{d}")
                            nc.vector.tensor_copy(ydb[:, :nwb], pyb[:, :nwb])
                            nc.sync.dma_start(yTs[s][d * P:(d + 1) * P, slb], ydb[:, :nwb])
    nc.finalize()
    return nc


def _route(xf: np.ndarray, w_router: np.ndarray):
    """Host router: softmax probs (float64 for stable ordering), top-2
    indices and renormalized combine weights, aux loss."""
    logits = xf.astype(np.float64) @ w_router.astype(np.float64)
    z = logits - logits.max(axis=-1, keepdims=True)
    p = np.exp(z)
    p /= p.sum(axis=-1, keepdims=True)

    ar = np.arange(xf.shape[0])
    top1 = p.argmax(axis=-1)
    pm = p.copy()
    pm[ar, top1] = -np.inf
    top2 = pm.argmax(axis=-1)
    p1 = p[ar, top1]
    p2 = p[ar, top2]
    c1 = p1 / (p1 + p2)
    c2 = p2 / (p1 + p2)

    tokens_per_expert = p.mean(axis=0)
    aux = AUX_COEFF * np.mean((tokens_per_expert - 1.0 / N_EXPERTS) ** 2)
    return top1, top2, c1, c2, np.float32(aux)


def kernel(x, w_router, w1, w2):
    global LAST_RESULTS
    x = np.asarray(x, dtype=np.float32)
    w_router = np.asarray(w_router, dtype=np.float32)
    w1 = np.asarray(w1, dtype=np.float32)
    w2 = np.asarray(w2, dtype=np.float32)

    xf = x.reshape(N_TOKENS, D)
    top1, top2, c1, c2, aux = _route(xf, w_router)

    # Gather tokens per expert.
    idx = [np.where((top1 == e) | (top2 == e))[0] for e in range(N_EXPERTS)]
    comb = [
        np.where(top1[idx[e]] == e, c1[idx[e]], c2[idx[e]]).astype(np.float32)
        for e in range(N_EXPERTS)
    ]
    counts = [len(i) for i in idx]

    # Sort experts by load; slot j serves expert rank 2j + (core // 4) with
    # hidden quarter (core % 4). Slot capacity = max count of its 2 experts,
    # rounded to 16 elements (64 B DMA row alignment).
    eorder = sorted(range(N_EXPERTS), key=lambda e: -counts[e])
    slot_widths = []
    for j in range(S_PER_CORE):
        w = max(counts[eorder[2 * j]], counts[eorder[2 * j + 1]], 1)
        slot_widths.append(max(128, -(-w // 16) * 16))
    slot_widths = tuple(slot_widths)

    xT_e = {}
    for j in range(S_PER_CORE):
        for g in range(2):
            e = eorder[2 * j + g]
            gbuf = np.zeros((D, slot_widths[j]), dtype=np.float32)
            gbuf[:, :counts[e]] = xf[idx[e]].T
            xT_e[e] = gbuf

    in_maps = []
    for core in range(N_CORES):
        g, q = divmod(core, N_SPLIT)
        w1c = np.empty((S_PER_CORE, D, H_SH), dtype=np.float32)
        w2c = np.empty((S_PER_CORE, H_SH, D), dtype=np.float32)
        im = {}
        for j in range(S_PER_CORE):
            e = eorder[2 * j + g]
            im[f"xT{j}"] = xT_e[e]
            w1c[j] = w1[e][:, q * H_SH:(q + 1) * H_SH]
            w2c[j] = w2[e][q * H_SH:(q + 1) * H_SH, :]
        im["w1s"] = w1c
        im["w2s"] = w2c
        in_maps.append(im)

    nc = _build(slot_widths)
    res = None
    for attempt in range(3):
        try:
            res = run_bass_kernel_spmd(nc, in_maps, core_ids=list(range(N_CORES)))
            break
        except Exception:
            if attempt == 2:
                raise
            time.sleep(5.0)
    LAST_RESULTS = res

    out = np.zeros((N_TOKENS, D), dtype=np.float32)
    for j in range(S_PER_CORE):
        for g in range(2):
            e = eorder[2 * j + g]
            acc = np.zeros((D, counts[e]), dtype=np.float32)
            for q in range(N_SPLIT):
                core = g * N_SPLIT + q
                acc += res.results[core][f"yT{j}"][:, :counts[e]]
            out[idx[e]] += comb[e][:, None] * acc.T

    return out.reshape(B, T, D), aux


# revision 24
# speedup vs baseline: 1.0387x; 1.0014x over previous
"""MoE top-2 routing kernel for 8 Trainium2 NeuronCores.

Strategy (expert-parallel with host-side dispatch):
  - Router (x @ w_router, softmax, top-2, combine weights) computed on host:
    it is 0.1% of the FLOPs and produces the dispatch indices needed to
    shard the tokens anyway.
  - Each of the 6 experts' FFN (D=1024 -> H=4096 -> D=1024) is split 4-ways
    along the hidden dim H into 24 shards of (1024 -> 1024 -> 1024).
    24 shards / 8 cores = 3 shards (slots) per core, perfectly
    weight-balanced. Experts are sorted by routed-token count and packed two
    per slot so each slot's capacity is the max of just those two experts
    (not the global max), minimizing zero-padding.
  - Tokens routed to expert e (gathered, transposed to [D, W] feature-major,
    zero-padded to the slot capacity W) are processed by all 4 of e's
    shards; each shard produces a partial y^T[D, W] (sum over its H
    quarter). Host sums the 4 partials per expert, scales by the top-2
    combine weight and scatter-adds into the output.
  - Device kernel per core: 3x dense fused MLP: h^T = gelu(w1s^T x^T) tile
    by tile, y^T = w2s^T h^T. Data is float32 in HBM but fed to the PE as
    float32r (single-pass ~fp22 matmul, fp32 PSUM accumulate) — the default
    float32 matmul mode (LOW_HIGH) is 2 passes and ~3x slower.

Measured on trn2 (8 cores): ~515 us HW exec (max core; ~513 us mean, with
occasional thermal-throttle runs up to ~580), out rel-err ~2.4e-4 vs the
fp32 jax reference. PE streams at ~213-227 ns per 128x128x512 matmul in
steady state; ~85% of the dense-PE roofline for the routed FLOPs. Fixed
overheads: ~10 us Tile preamble, ~9 us kernel-tail drain/EVSEM epilogue,
~18 us cold-start weight DMA.
"""

import functools
import sys
import time
import types

import numpy as np

# bass_utils' axon trace path does `from antenv.axon_hooks import ...`; some
# images ship an antenv without that module, which would turn a BASS_TRACE=1
# run into an ImportError. Register a no-op stand-in so tracing degrades
# gracefully instead (run_bass_kernel_spmd skips tracing on a None hook).
try:
    import antenv.axon_hooks  # noqa: F401
except Exception:
    try:
        import antenv  # noqa: F401
        _hooks = types.ModuleType("antenv.axon_hooks")
        _hooks._HOOK = None
        _hooks.set_axon_ntff_profile_hook = (
            lambda hook: setattr(_hooks, "_HOOK", hook)
        )
        _hooks.get_axon_ntff_profile_hook = lambda: _hooks._HOOK
        sys.modules["antenv.axon_hooks"] = _hooks
        antenv.axon_hooks = _hooks
    except Exception:
        pass

import concourse.bacc as bacc
import concourse.bass as bass
import concourse.mybir as mybir
import concourse.tile as tile
from concourse.bass_utils import run_bass_kernel_spmd

N_EXPERTS = 6
TOP_K = 2
AUX_COEFF = 0.01
B, T, D, H = 4, 2048, 1024, 4096
N_TOKENS = B * T
N_CORES = 8
N_SPLIT = 4                       # H split per expert
H_SH = H // N_SPLIT               # 1024
S_PER_CORE = 3                    # expert-shard slots per core
P = 128
FREE = 512                        # matmul moving free dim / PSUM bank width

# Populated by kernel() with the BassKernelResults of the last device run so
# a test harness can read exec_time_ns when BASS_TRACE=1 is set.
LAST_RESULTS = None


def _tile_widths(W: int) -> list:
    widths = [FREE] * (W // FREE)
    if W % FREE:
        widths.append(W % FREE)
    return widths


@functools.cache
def _build(slot_widths: tuple) -> bass.Bass:
    """Bass program for one core: 3 independent (1024 -> 1024 -> 1024) dense
    MLP shards, slot j over slot_widths[j] tokens, float32r feature-major."""
    f32 = mybir.dt.float32
    f32r = mybir.dt.float32r
    KD = D // P      # 8 contraction chunks for layer 1
    KH = H_SH // P   # 8 contraction chunks for layer 2

    nc = bacc.Bacc()
    xTs = [nc.dram_tensor(f"xT{j}", [D, W], f32r, kind="ExternalInput")
           for j, W in enumerate(slot_widths)]
    w1s = nc.dram_tensor("w1s", [S_PER_CORE, D, H_SH], f32r, kind="ExternalInput")
    w2s = nc.dram_tensor("w2s", [S_PER_CORE, H_SH, D], f32r, kind="ExternalInput")
    yTs = [nc.dram_tensor(f"yT{j}", [D, W], f32r, kind="ExternalOutput")
           for j, W in enumerate(slot_widths)]

    with tile.TileContext(nc) as tc:
        with (
            tc.tile_pool(name="w1p", bufs=3 * KD) as w1p,
            tc.tile_pool(name="w2p", bufs=KH + KH // 2) as w2p,
            tc.tile_pool(name="xp", bufs=2 * KD + 2) as xp,
            tc.tile_pool(name="hp", bufs=2 * KH + 2) as hp,
            tc.tile_pool(name="yp", bufs=4) as yp,
            tc.tile_pool(name="psh", bufs=4, space="PSUM") as psh,
            tc.tile_pool(name="psy", bufs=4, space="PSUM") as psy,
        ):
            def load_x(s, n, nsl, nw):
                xt = []
                for k in range(KD):
                    xk = xp.tile([P, FREE], f32r, tag="x", name=f"x_{s}_{n}_{k}")
                    nc.sync.dma_start(xk[:, :nw], xTs[s][k * P:(k + 1) * P, nsl])
                    xt.append(xk)
                return xt

            for s in range(S_PER_CORE):
                widths = _tile_widths(slot_widths[s])
                # Pair the n-tiles: two subtiles share one weight slice per
                # (m, k), halving LDWEIGHTS pressure on the PE. Each pair is
                # (col offset a, width a, col offset b | None, width b).
                offs = [0]
                for w in widths:
                    offs.append(offs[-1] + w)
                pairs = []
                i = 0
                while i < len(widths):
                    if i + 1 < len(widths):
                        pairs.append((offs[i], widths[i], offs[i + 1], widths[i + 1]))
                        i += 2
                    else:
                        pairs.append((offs[i], widths[i], None, 0))
                        i += 1

                # w1 loaded in half-width tiles, low halves first: the first
                # matmuls (m=0..3) need only cols 0:512 of each k chunk, so
                # compute starts after 2 MB instead of 4 MB of weight DMA.
                w1t = [[None, None] for _ in range(KD)]

                def load_w1_half(s, half):
                    csl = slice(half * (H_SH // 2), (half + 1) * (H_SH // 2))
                    for k in range(KD):
                        w1k = w1p.tile([P, H_SH // 2], f32r, tag="w1",
                                       name=f"w1_{s}_{k}_{half}")
                        nc.sync.dma_start(w1k[:], w1s[s, k * P:(k + 1) * P, csl])
                        w1t[k][half] = w1k

                # DMA issue order tracks first-use order: w1 low halves
                # (m=0..3), first n-pair's x, w1 high halves, then w2
                # (layer 2 starts ~15 us after layer 1).
                load_w1_half(s, 0)
                ca0, nwa0, cb0, nwb0 = pairs[0]
                xta0 = load_x(s, ca0, slice(ca0, ca0 + nwa0), nwa0)
                xtb0 = (load_x(s, cb0, slice(cb0, cb0 + nwb0), nwb0)
                        if cb0 is not None else None)
                load_w1_half(s, 1)
                w2t = []
                for k in range(KH):
                    w2k = w2p.tile([P, D], f32r, tag="w2", name=f"w2_{s}_{k}")
                    nc.sync.dma_start(w2k[:], w2s[s, k * P:(k + 1) * P, :])
                    w2t.append(w2k)

                for pi, (ca, nwa, cb, nwb) in enumerate(pairs):
                    na = ca
                    sla = slice(ca, ca + nwa)
                    slb = slice(cb, cb + nwb) if cb is not None else None
                    if pi == 0:
                        xta, xtb = xta0, xtb0
                    else:
                        xta = load_x(s, ca, sla, nwa)
                        xtb = load_x(s, cb, slb, nwb) if cb is not None else None
                    hta, htb = [], []
                    for m in range(KH):
                        pha = psh.tile([P, FREE], f32, tag="ps_h", name=f"pha_{s}_{na}_{m}")
                        phb = (psh.tile([P, FREE], f32, tag="ps_h", name=f"phb_{s}_{na}_{m}")
                               if xtb is not None else None)
                        for k in range(KD):
                            w1sl = w1t[k][m // 4][:, (m % 4) * P:(m % 4 + 1) * P]
                            nc.tensor.matmul(pha[:, :nwa], w1sl, xta[k][:, :nwa],
                                             start=(k == 0), stop=(k == KD - 1))
                            if phb is not None:
                                nc.tensor.matmul(phb[:, :nwb], w1sl, xtb[k][:, :nwb],
                                                 start=(k == 0), stop=(k == KD - 1))
                        hma = hp.tile([P, FREE], f32r, tag="h", name=f"hma_{s}_{na}_{m}")
                        nc.scalar.activation(hma[:, :nwa], pha[:, :nwa],
                                             mybir.ActivationFunctionType.Gelu)
                        hta.append(hma)
                        if phb is not None:
                            hmb = hp.tile([P, FREE], f32r, tag="h", name=f"hmb_{s}_{na}_{m}")
                            nc.scalar.activation(hmb[:, :nwb], phb[:, :nwb],
                                                 mybir.ActivationFunctionType.Gelu)
                            htb.append(hmb)
                    for d in range(KD):
                        pya = psy.tile([P, FREE], f32, tag="ps_y", name=f"pya_{s}_{na}_{d}")
                        pyb = (psy.tile([P, FREE], f32, tag="ps_y", name=f"pyb_{s}_{na}_{d}")
                               if xtb is not None else None)
                        for k in range(KH):
                            w2sl = w2t[k][:, d * P:(d + 1) * P]
                            nc.tensor.matmul(pya[:, :nwa], w2sl, hta[k][:, :nwa],
                                             start=(k == 0), stop=(k == KH - 1))
                            if pyb is not None:
                                nc.tensor.matmul(pyb[:, :nwb], w2sl, htb[k][:, :nwb],
                                                 start=(k == 0), stop=(k == KH - 1))
                        yda = yp.tile([P, FREE], f32r, tag="y", name=f"yda_{s}_{na}_{d}")
                        nc.vector.tensor_copy(yda[:, :nwa], pya[:, :nwa])
                        nc.sync.dma_start(yTs[s][d * P:(d + 1) * P, sla], yda[:, :nwa])
                        if pyb is not None:
                            ydb = yp.tile([P, FREE], f32r, tag="y", name=f"ydb_{s}_{na}_{d}")
                            nc.vector.tensor_copy(ydb[:, :nwb], pyb[:, :nwb])
                            nc.sync.dma_start(yTs[s][d * P:(d + 1) * P, slb], ydb[:, :nwb])
    nc.finalize()
    return nc


def _route(xf: np.ndarray, w_router: np.ndarray):
    """Host router: softmax probs (float64 for stable ordering), top-2
    indices and renormalized combine weights, aux loss."""
    logits = xf.astype(np.float64) @ w_router.astype(np.float64)
    z = logits - logits.max(axis=-1, keepdims=True)
    p = np.exp(z)
    p /= p.sum(axis=-1, keepdims=True)

    ar = np.arange(xf.shape[0])
    top1 = p.argmax(axis=-1)
    pm = p.copy()
    pm[ar, top1] = -np.inf
    top2 = pm.argmax(axis=-1)
    p1 = p[ar, top1]
    p2 = p[ar, top2]
    c1 = p1 / (p1 + p2)
    c2 = p2 / (p1 + p2)

    tokens_per_expert = p.mean(axis=0)
    aux = AUX_COEFF * np.mean((tokens_per_expert - 1.0 / N_EXPERTS) ** 2)
    return top1, top2, c1, c2, np.float32(aux)


def kernel(x, w_router, w1, w2):
    global LAST_RESULTS
    x = np.asarray(x, dtype=np.float32)
    w_router = np.asarray(w_router, dtype=np.float32)
    w1 = np.asarray(w1, dtype=np.float32)
    w2 = np.asarray(w2, dtype=np.float32)

    xf = x.reshape(N_TOKENS, D)
    top1, top2, c1, c2, aux = _route(xf, w_router)

    # Gather tokens per expert.
    idx = [np.where((top1 == e) | (top2 == e))[0] for e in range(N_EXPERTS)]
    comb = [
        np.where(top1[idx[e]] == e, c1[idx[e]], c2[idx[e]]).astype(np.float32)
        for e in range(N_EXPERTS)
    ]
    counts = [len(i) for i in idx]

    # Sort experts by load; slot j serves expert rank 2j + (core // 4) with
    # hidden quarter (core % 4). Slot capacity = max count of its 2 experts,
    # rounded to 16 elements (64 B DMA row alignment).
    eorder = sorted(range(N_EXPERTS), key=lambda e: -counts[e])
    slot_widths = []
    for j in range(S_PER_CORE):
        w = max(counts[eorder[2 * j]], counts[eorder[2 * j + 1]], 1)
        slot_widths.append(max(128, -(-w // 16) * 16))
    slot_widths = tuple(slot_widths)

    xT_e = {}
    for j in range(S_PER_CORE):
        for g in range(2):
            e = eorder[2 * j + g]
            gbuf = np.zeros((D, slot_widths[j]), dtype=np.float32)
            gbuf[:, :counts[e]] = xf[idx[e]].T
            xT_e[e] = gbuf

    in_maps = []
    for core in range(N_CORES):
        g, q = divmod(core, N_SPLIT)
        w1c = np.empty((S_PER_CORE, D, H_SH), dtype=np.float32)
        w2c = np.empty((S_PER_CORE, H_SH, D), dtype=np.float32)
        im = {}
        for j in range(S_PER_CORE):
            e = eorder[2 * j + g]
            im[f"xT{j}"] = xT_e[e]
            w1c[j] = w1[e][:, q * H_SH:(q + 1) * H_SH]
            w2c[j] = w2[e][q * H_SH:(q + 1) * H_SH, :]
        im["w1s"] = w1c
        im["w2s"] = w2c
        in_maps.append(im)

    nc = _build(slot_widths)
    res = None
    for attempt in range(3):
        try:
            res = run_bass_kernel_spmd(nc, in_maps, core_ids=list(range(N_CORES)))
            break
        except Exception:
            if attempt == 2:
                raise
            time.sleep(5.0)
    LAST_RESULTS = res

    out = np.zeros((N_TOKENS, D), dtype=np.float32)
    for j in range(S_PER_CORE):
        for g in range(2):
            e = eorder[2 * j + g]
            acc = np.zeros((D, counts[e]), dtype=np.float32)
            for q in range(N_SPLIT):
                core = g * N_SPLIT + q
                acc += res.results[core][f"yT{j}"][:, :counts[e]]
            out[idx[e]] += comb[e][:, None] * acc.T

    return out.reshape(B, T, D), aux
